# revision 29
# baseline (speedup 1.0000x reference)
"""Deep Neural Decision Forest (DNDF) forward on 8 Trainium2 NeuronCores.

Data-parallel over batch: each core gets 1024 of 8192 rows of x; W/b/leafs
replicated. Per core:
  net^T-free layout: tree_net = x @ W.T  (f32r matmuls, full PE rate)
  p = sigmoid(net) [ACT], q = 1-p [GPSIMD], both bf16
  leaf path probs via 4 levels of contiguous bf16 muls [DVE] - W rows are
  host-permuted (level-major, bit-reversed within level, tree innermost) so
  every gate read is contiguous
  routes [128b, 2048f] -> DMA-xbar transpose (bf16) -> [2048f, 1024b]
  out^T[16, 1024] = (softmax(leafs)/64)^T-ish via PE with leafs as lhsT
Host gathers the 8 out^T shards and transposes back.
"""
import sys

for _p in ("/opt/trn_rl_repo",):
    if _p not in sys.path:
        sys.path.append(_p)

import numpy as np

NCORES = 8
B, K, T, I, NL, O = 8192, 512, 64, 31, 32, 16
BSH = B // NCORES          # 1024 batch rows per core
BT = BSH // 128            # 8 batch tiles per core
NT = 4                     # n-tiles of 496 = 31*16 over the 1984 internals
NW = 496
F = T * NL                 # 2048 leaf features

_CACHE = {}


def _bitrev(x, bits):
    r = 0
    for _ in range(bits):
        r = (r << 1) | (x & 1)
        x >>= 1
    return r


def _perm_nodes():
    out = []
    for d in range(5):
        for r in range(2 ** d):
            out.append(2 ** d - 1 + _bitrev(r, d))
    return np.array(out)


_PERM = _perm_nodes()                                     # slot -> node
_LEAF_PERM = np.array([_bitrev(i, 5) for i in range(NL)])  # layout -> leaf


def _split_sync_waits(nc, cap=1):
    """This container's walrus rejects >1 sync wait per instruction; move
    excess waits onto standalone EventSemaphore insts (same-engine order
    makes this equivalent)."""
    import bass_rust
    from concourse import mybir

    n = 0
    for fn in nc.m.functions:
        for bb in fn.blocks:
            insts = bb.instructions
            out = []
            changed = False
            for ins in insts:
                si = getattr(ins, "sync_info", None)
                waits = list(si.on_wait) if si is not None and si.on_wait else []
                if len(waits) > cap:
                    changed = True
                    for k in range(0, len(waits) - cap, cap):
                        n += 1
                        out.append(mybir.InstEventSemaphore(
                            name=f"I-wsplit{n}",
                            engine=ins.engine,
                            ins=[], outs=[],
                            debug=ins.debug,
                            sync_info=bass_rust.SyncInfo(
                                on_wait=waits[k:k + cap], on_update=[]),
                        ))
                    ins.sync_info = bass_rust.SyncInfo(
                        on_wait=waits[len(waits) - cap:],
                        on_update=list(si.on_update))
                out.append(ins)
            if changed:
                insts[:] = out


def _build(use_b):
    import concourse.bass as bass
    import concourse.tile as tile
    from concourse import mybir

    F32 = mybir.dt.float32
    F32R = mybir.dt.float32r
    BF16 = mybir.dt.bfloat16
    AF = mybir.ActivationFunctionType
    MUL = mybir.AluOpType.mult
    ADD = mybir.AluOpType.add

    nc = bass.Bass()
    xt_d = nc.dram_tensor("xt", [K, BSH], BF16, kind="ExternalInput")
    wt_d = nc.dram_tensor("wt", [K, T * I], BF16, kind="ExternalInput")
    lf_d = nc.dram_tensor("lf", [F, O], F32, kind="ExternalInput")
    if use_b:
        b_d = nc.dram_tensor("bb", [1, T * I], BF16, kind="ExternalInput")
    ot_d = nc.dram_tensor("ot", [O, BSH], F32, kind="ExternalOutput")

    with tile.TileContext(nc) as tc:
        with (
            tc.tile_pool(name="const", bufs=1) as const,
            tc.tile_pool(name="work", bufs=2) as work,
            tc.tile_pool(name="psum", bufs=8, space="PSUM") as psum,
        ):
            # ---- prologue loads (f32 bits into f32r tiles via bitcast),
            # chunked per K-tile so mm1 can start after the first pair ----
            xt = const.tile([128, 4, BSH], BF16)
            wt = const.tile([128, 4, T * I], BF16)
            for c in range(4):
                nc.sync.dma_start(
                    xt[:, c, :], xt_d[c * 128:(c + 1) * 128, :])
                for h in range(2):
                    nc.sync.dma_start(
                        wt[:, c, h * 992:(h + 1) * 992],
                        wt_d[c * 128:(c + 1) * 128, h * 992:(h + 1) * 992])
            if use_b:
                bt_s = const.tile([1, T * I], BF16)
                nc.sync.dma_start(bt_s[:], b_d[:])
                ones = const.tile([1, 128], BF16)
                nc.gpsimd.memset(ones[:], 1.0)

            # ---- leaf softmax (off critical path) ----
            lf = const.tile([128, 16, O], F32)
            nc.sync.dma_start(lf[:], lf_d.rearrange("(c k) o -> k c o", k=128))
            ex = const.tile([128, 16, O], F32)
            nc.scalar.activation(ex[:], lf[:], AF.Exp)
            sums = const.tile([128, 16], F32)
            nc.vector.reduce_sum(sums[:], ex[:], axis=mybir.AxisListType.X)
            nc.vector.tensor_scalar_mul(sums[:], sums[:], float(T))
            rs = const.tile([128, 16], F32)
            nc.vector.reciprocal(rs[:], sums[:])
            lsm = const.tile([128, 16, O], BF16)
            for c in range(16):
                nc.vector.tensor_scalar(
                    lsm[:, c, :], ex[:, c, :], rs[:, c:c + 1], None, op0=MUL)

            rT = const.tile([128, 16, BSH], BF16)   # routes^T, filled per btile

            # ---- main loop over batch tiles ----
            outT = const.tile([O, BSH], F32)

            def emit_mm2(nch):
                po = psum.tile([O, 512], F32, tag="ps")
                for kcc in range(16):
                    nc.tensor.matmul(
                        po[:], lsm[:, kcc, :],
                        rT[:, kcc, nch * 512:(nch + 1) * 512],
                        start=(kcc == 0), stop=(kcc == 15))
                nc.vector.tensor_copy(
                    outT[:, nch * 512:(nch + 1) * 512], po[:])

            def emit_b_row(banks):
                for nt in range(NT):
                    nc.tensor.matmul(
                        banks[nt][:, 0:NW], ones[:],
                        bt_s[:, nt * NW:(nt + 1) * NW],
                        start=True, stop=False)

            def emit_post(bt, banks):
                # per-bank sigmoid: bank nt's p-slice is ready as soon as its
                # accumulation stops, letting the routing chain start early
                p = work.tile([128, T * I], BF16, tag="p")
                for nt in range(NT):
                    nc.scalar.activation(
                        p[:, nt * NW:(nt + 1) * NW], banks[nt][:, 0:NW],
                        AF.Sigmoid)

                # q-branches via R*(1-p) = R - R*p: only the 64-wide level-0
                # q-slice is ever materialized
                q0 = work.tile([128, T], BF16, tag="q0")
                nc.vector.tensor_scalar(
                    q0[:], p[:, 0:T], -1.0, 1.0, op0=MUL, op1=ADD)

                R1 = work.tile([128, 2, 2 * T], BF16, tag="R1")
                R2 = work.tile([128, 2, 4 * T], BF16, tag="R2")
                R3 = work.tile([128, 2, 8 * T], BF16, tag="R3")
                R4 = work.tile([128, 2, 16 * T], BF16, tag="R4")
                # level 1, split so R0 = [p0|q0] never materializes
                nc.vector.tensor_mul(R1[:, 0, 0:T], p[:, 0:T], p[:, T:2 * T])
                nc.vector.tensor_mul(R1[:, 0, T:2 * T], q0[:], p[:, 2 * T:3 * T])
                nc.vector.tensor_sub(R1[:, 1, 0:T], p[:, 0:T], R1[:, 0, 0:T])
                nc.vector.tensor_sub(R1[:, 1, T:2 * T], q0[:], R1[:, 0, T:2 * T])
                prev = R1[:].rearrange("p a c -> p (a c)")
                for (R, s0, s1) in [(R2, 3, 7), (R3, 7, 15), (R4, 15, 31)]:
                    flat = R[:].rearrange("p a c -> p (a c)")
                    nc.vector.tensor_mul(            # p-branch (1-D, bf16 2x)
                        R[:, 0, :], prev, p[:, s0 * T:s1 * T])
                    nc.vector.tensor_sub(            # q-branch = prev - p-branch
                        R[:, 1, :], prev, R[:, 0, :])
                    prev = flat                 # 1-D view of full level

                nc.sync.dma_start_transpose(
                    rT[:, :, bt * 128:(bt + 1) * 128],
                    R4[:].rearrange("p a c -> p (a c)"))

            def emit_btile(bt):
                # nt-outer with one PSUM bank per n-tile: each bank frees
                # right after its own sigmoid, and early banks' sigmoids and
                # routing levels overlap the later banks' matmuls
                xcol = xt[:, :, bt * 128:(bt + 1) * 128]
                banks = []
                for nt in range(NT):
                    pb = psum.tile([128, 512], F32, tag="ps", name=f"pb{bt}_{nt}")
                    if use_b:
                        nc.tensor.matmul(
                            pb[:, 0:NW], ones[:],
                            bt_s[:, nt * NW:(nt + 1) * NW],
                            start=True, stop=False)
                    for kc in range(4):
                        nc.tensor.matmul(
                            pb[:, 0:NW],
                            xcol[:, kc, :],
                            wt[:, kc, nt * NW:(nt + 1) * NW],
                            start=(kc == 0) and not use_b,
                            stop=(kc == 3),
                        )
                    banks.append(pb)
                emit_post(bt, banks)

            # streaming prologue: batch tiles 0-1 chase the chunked input
            # DMAs (kc-outer, W-half-inner) so PE starts as soon as the
            # first K-chunk lands
            banks0 = [psum.tile([128, 512], F32, tag="ps", name=f"pbs0_{i}") for i in range(NT)]
            banks1 = [psum.tile([128, 512], F32, tag="ps", name=f"pbs1_{i}") for i in range(NT)]
            if use_b:
                emit_b_row(banks0)
                emit_b_row(banks1)
            for kc in range(4):
                if kc < 3:
                    order = [(h, bt) for h in range(2) for bt in range(2)]
                else:
                    # de-interleave the last K-sweep: bt0 finishes (and frees
                    # its PSUM banks via sigmoids) ahead of bt1
                    order = [(h, bt) for bt in range(2) for h in range(2)]
                for h, bt in order:
                    bks = banks0 if bt == 0 else banks1
                    xcol = xt[:, kc, bt * 128:(bt + 1) * 128]
                    for nt in (2 * h, 2 * h + 1):
                        nc.tensor.matmul(
                            bks[nt][:, 0:NW], xcol,
                            wt[:, kc, nt * NW:(nt + 1) * NW],
                            start=(kc == 0) and not use_b,
                            stop=(kc == 3))
            emit_post(0, banks0)
            emit_post(1, banks1)

            # interleave: mm2 half 0 only needs batch tiles 0-3
            for bt in range(2, 7):
                emit_btile(bt)
            emit_mm2(0)
            emit_btile(7)
            emit_mm2(1)
            nc.sync.dma_start(ot_d[:], outT[:])

    _split_sync_waits(nc)
    return nc


def kernel(x, W, b, leafs):
    import ml_dtypes
    x = np.asarray(x, dtype=np.float32)
    W = np.asarray(W, dtype=np.float32)
    b = np.asarray(b, dtype=np.float32)
    leafs = np.asarray(leafs, dtype=np.float32)
    use_b = bool(np.any(b))

    # host-side layout prep (permutation / transposition / bf16 rounding)
    BF = ml_dtypes.bfloat16
    Wp = W.reshape(T, I, K)[:, _PERM, :].transpose(1, 0, 2).reshape(T * I, K)
    wt_np = np.ascontiguousarray(Wp.T.astype(BF))          # [K, 1984] bf16
    lf_np = np.ascontiguousarray(
        leafs[:, _LEAF_PERM, :].transpose(1, 0, 2).reshape(F, O))
    xT = x.T.astype(BF)                                    # [K, B] bf16
    in_maps = []
    for c in range(NCORES):
        m = {
            "xt": np.ascontiguousarray(xT[:, c * BSH:(c + 1) * BSH]),
            "wt": wt_np,
            "lf": lf_np,
        }
        if use_b:
            bp = b.reshape(T, I)[:, _PERM].T.reshape(1, T * I)
            m["bb"] = np.ascontiguousarray(bp.astype(BF))
        in_maps.append(m)

    key = ("nc", use_b)
    if key not in _CACHE:
        _CACHE[key] = _build(use_b)
    nc = _CACHE[key]

    from concourse.bass_utils import run_bass_kernel_spmd
    res = run_bass_kernel_spmd(nc, in_maps, list(range(NCORES)))
    out = np.concatenate(
        [res.results[c]["ot"].T for c in range(NCORES)], axis=0)
    return np.ascontiguousarray(out.astype(np.float32))


if __name__ == "__main__":
    rng = np.random.default_rng(0)
    ins = {
        "x": rng.standard_normal((B, K)).astype(np.float32),
        "W": (rng.standard_normal((T * I, K)) / np.sqrt(K)).astype(np.float32),
        "b": np.zeros(T * I, np.float32),
        "leafs": rng.standard_normal((T, NL, O)).astype(np.float32),
    }
    print(kernel(**ins)[:2, :4])


# revision 30
# speedup vs baseline: 1.1449x; 1.1449x over previous
"""Deep Neural Decision Forest (DNDF) forward on 8 Trainium2 NeuronCores.

Data-parallel over batch: each core gets 1024 of 8192 rows of x; W/b/leafs
replicated. Per core:
  net^T-free layout: tree_net = x @ W.T  (f32r matmuls, full PE rate)
  p = sigmoid(net) [ACT], q = 1-p [GPSIMD], both bf16
  leaf path probs via 4 levels of contiguous bf16 muls [DVE] - W rows are
  host-permuted (level-major, bit-reversed within level, tree innermost) so
  every gate read is contiguous
  routes [128b, 2048f] -> DMA-xbar transpose (bf16) -> [2048f, 1024b]
  out^T[16, 1024] = (softmax(leafs)/64)^T-ish via PE with leafs as lhsT
Host gathers the 8 out^T shards and transposes back.
"""
import sys

for _p in ("/opt/trn_rl_repo",):
    if _p not in sys.path:
        sys.path.append(_p)

import numpy as np

NCORES = 8
B, K, T, I, NL, O = 8192, 512, 64, 31, 32, 16
BSH = B // NCORES          # 1024 batch rows per core
BT = BSH // 128            # 8 batch tiles per core
NT = 4                     # n-tiles of 496 = 31*16 over the 1984 internals
NW = 496
F = T * NL                 # 2048 leaf features

_CACHE = {}


def _bitrev(x, bits):
    r = 0
    for _ in range(bits):
        r = (r << 1) | (x & 1)
        x >>= 1
    return r


def _perm_nodes():
    out = []
    for d in range(5):
        for r in range(2 ** d):
            out.append(2 ** d - 1 + _bitrev(r, d))
    return np.array(out)


_PERM = _perm_nodes()                                     # slot -> node
_LEAF_PERM = np.array([_bitrev(i, 5) for i in range(NL)])  # layout -> leaf


def _split_sync_waits(nc, cap=1):
    """This container's walrus rejects >1 sync wait per instruction; move
    excess waits onto standalone EventSemaphore insts (same-engine order
    makes this equivalent)."""
    import bass_rust
    from concourse import mybir

    n = 0
    for fn in nc.m.functions:
        for bb in fn.blocks:
            insts = bb.instructions
            out = []
            changed = False
            for ins in insts:
                si = getattr(ins, "sync_info", None)
                waits = list(si.on_wait) if si is not None and si.on_wait else []
                if len(waits) > cap:
                    changed = True
                    for k in range(0, len(waits) - cap, cap):
                        n += 1
                        out.append(mybir.InstEventSemaphore(
                            name=f"I-wsplit{n}",
                            engine=ins.engine,
                            ins=[], outs=[],
                            debug=ins.debug,
                            sync_info=bass_rust.SyncInfo(
                                on_wait=waits[k:k + cap], on_update=[]),
                        ))
                    ins.sync_info = bass_rust.SyncInfo(
                        on_wait=waits[len(waits) - cap:],
                        on_update=list(si.on_update))
                out.append(ins)
            if changed:
                insts[:] = out


def _build(use_b):
    import concourse.bass as bass
    import concourse.tile as tile
    from concourse import mybir

    F32 = mybir.dt.float32
    F32R = mybir.dt.float32r
    BF16 = mybir.dt.bfloat16
    AF = mybir.ActivationFunctionType
    MUL = mybir.AluOpType.mult
    ADD = mybir.AluOpType.add

    nc = bass.Bass()
    xt_d = nc.dram_tensor("xt", [K, BSH], BF16, kind="ExternalInput")
    wt_d = nc.dram_tensor("wt", [K, T * I], BF16, kind="ExternalInput")
    lf_d = nc.dram_tensor("lf", [F, O], F32, kind="ExternalInput")
    if use_b:
        b_d = nc.dram_tensor("bb", [1, T * I], BF16, kind="ExternalInput")
    ot_d = nc.dram_tensor("ot", [O, BSH], F32, kind="ExternalOutput")

    with tile.TileContext(nc) as tc:
        with (
            tc.tile_pool(name="const", bufs=1) as const,
            tc.tile_pool(name="work", bufs=2) as work,
            tc.tile_pool(name="psum", bufs=8, space="PSUM") as psum,
        ):
            # ---- prologue loads (f32 bits into f32r tiles via bitcast),
            # chunked per K-tile so mm1 can start after the first pair ----
            xt = const.tile([128, 4, BSH], BF16)
            wt = const.tile([128, 4, T * I], BF16)
            for c in range(4):
                nc.sync.dma_start(
                    xt[:, c, :], xt_d[c * 128:(c + 1) * 128, :])
                for h in range(2):
                    nc.sync.dma_start(
                        wt[:, c, h * 992:(h + 1) * 992],
                        wt_d[c * 128:(c + 1) * 128, h * 992:(h + 1) * 992])
            if use_b:
                bt_s = const.tile([1, T * I], BF16)
                nc.sync.dma_start(bt_s[:], b_d[:])
                ones = const.tile([1, 128], BF16)
                nc.gpsimd.memset(ones[:], 1.0)

            # ---- leaf softmax (off critical path) ----
            lf = const.tile([128, 16, O], F32)
            nc.sync.dma_start(lf[:], lf_d.rearrange("(c k) o -> k c o", k=128))
            ex = const.tile([128, 16, O], F32)
            nc.scalar.activation(ex[:], lf[:], AF.Exp)
            sums = const.tile([128, 16], F32)
            nc.vector.reduce_sum(sums[:], ex[:], axis=mybir.AxisListType.X)
            nc.vector.tensor_scalar_mul(sums[:], sums[:], float(T))
            rs = const.tile([128, 16], F32)
            nc.vector.reciprocal(rs[:], sums[:])
            lsm = const.tile([128, 16, O], BF16)
            for c in range(16):
                nc.gpsimd.tensor_scalar(
                    lsm[:, c, :], ex[:, c, :], rs[:, c:c + 1], None, op0=MUL)

            rT = const.tile([128, 16, BSH], BF16)   # routes^T, filled per btile

            # ---- main loop over batch tiles ----
            outT = const.tile([O, BSH], F32)

            def emit_mm2(nch):
                po = psum.tile([O, 512], F32, tag="ps")
                for kcc in range(16):
                    nc.tensor.matmul(
                        po[:], lsm[:, kcc, :],
                        rT[:, kcc, nch * 512:(nch + 1) * 512],
                        start=(kcc == 0), stop=(kcc == 15))
                nc.vector.tensor_copy(
                    outT[:, nch * 512:(nch + 1) * 512], po[:])

            def emit_b_row(banks):
                for nt in range(NT):
                    nc.tensor.matmul(
                        banks[nt][:, 0:NW], ones[:],
                        bt_s[:, nt * NW:(nt + 1) * NW],
                        start=True, stop=False)

            def emit_post(bt, banks):
                # per-bank sigmoid: bank nt's p-slice is ready as soon as its
                # accumulation stops, letting the routing chain start early
                p = work.tile([128, T * I], BF16, tag="p")
                for nt in range(NT):
                    nc.scalar.activation(
                        p[:, nt * NW:(nt + 1) * NW], banks[nt][:, 0:NW],
                        AF.Sigmoid)

                # q-branches via R*(1-p) = R - R*p: only the 64-wide level-0
                # q-slice is ever materialized
                q0 = work.tile([128, T], BF16, tag="q0")
                nc.vector.tensor_scalar(
                    q0[:], p[:, 0:T], -1.0, 1.0, op0=MUL, op1=ADD)

                R1 = work.tile([128, 2, 2 * T], BF16, tag="R1")
                R2 = work.tile([128, 2, 4 * T], BF16, tag="R2")
                R3 = work.tile([128, 2, 8 * T], BF16, tag="R3")
                R4 = work.tile([128, 2, 16 * T], BF16, tag="R4")
                # level 1, split so R0 = [p0|q0] never materializes
                nc.vector.tensor_mul(R1[:, 0, 0:T], p[:, 0:T], p[:, T:2 * T])
                nc.vector.tensor_mul(R1[:, 0, T:2 * T], q0[:], p[:, 2 * T:3 * T])
                nc.vector.tensor_sub(R1[:, 1, 0:T], p[:, 0:T], R1[:, 0, 0:T])
                nc.vector.tensor_sub(R1[:, 1, T:2 * T], q0[:], R1[:, 0, T:2 * T])
                prev = R1[:].rearrange("p a c -> p (a c)")
                for (R, s0, s1) in [(R2, 3, 7), (R3, 7, 15), (R4, 15, 31)]:
                    flat = R[:].rearrange("p a c -> p (a c)")
                    nc.vector.tensor_mul(            # p-branch (1-D, bf16 2x)
                        R[:, 0, :], prev, p[:, s0 * T:s1 * T])
                    nc.vector.tensor_sub(            # q-branch = prev - p-branch
                        R[:, 1, :], prev, R[:, 0, :])
                    prev = flat                 # 1-D view of full level

                nc.sync.dma_start_transpose(
                    rT[:, :, bt * 128:(bt + 1) * 128],
                    R4[:].rearrange("p a c -> p (a c)"))

            def emit_btile(bt):
                # nt-outer with one PSUM bank per n-tile: each bank frees
                # right after its own sigmoid, and early banks' sigmoids and
                # routing levels overlap the later banks' matmuls
                xcol = xt[:, :, bt * 128:(bt + 1) * 128]
                banks = []
                for nt in range(NT):
                    pb = psum.tile([128, 512], F32, tag="ps", name=f"pb{bt}_{nt}")
                    if use_b:
                        nc.tensor.matmul(
                            pb[:, 0:NW], ones[:],
                            bt_s[:, nt * NW:(nt + 1) * NW],
                            start=True, stop=False)
                    for kc in range(4):
                        nc.tensor.matmul(
                            pb[:, 0:NW],
                            xcol[:, kc, :],
                            wt[:, kc, nt * NW:(nt + 1) * NW],
                            start=(kc == 0) and not use_b,
                            stop=(kc == 3),
                        )
                    banks.append(pb)
                emit_post(bt, banks)

            # streaming prologue: batch tiles 0-1 chase the chunked input
            # DMAs (kc-outer, W-half-inner) so PE starts as soon as the
            # first K-chunk lands
            banks0 = [psum.tile([128, 512], F32, tag="ps", name=f"pbs0_{i}") for i in range(NT)]
            banks1 = [psum.tile([128, 512], F32, tag="ps", name=f"pbs1_{i}") for i in range(NT)]
            if use_b:
                emit_b_row(banks0)
                emit_b_row(banks1)
            for kc in range(4):
                if kc < 3:
                    order = [(h, bt) for h in range(2) for bt in range(2)]
                else:
                    # de-interleave the last K-sweep: bt0 finishes (and frees
                    # its PSUM banks via sigmoids) ahead of bt1
                    order = [(h, bt) for bt in range(2) for h in range(2)]
                for h, bt in order:
                    bks = banks0 if bt == 0 else banks1
                    xcol = xt[:, kc, bt * 128:(bt + 1) * 128]
                    for nt in (2 * h, 2 * h + 1):
                        nc.tensor.matmul(
                            bks[nt][:, 0:NW], xcol,
                            wt[:, kc, nt * NW:(nt + 1) * NW],
                            start=(kc == 0) and not use_b,
                            stop=(kc == 3))
            emit_post(0, banks0)
            emit_post(1, banks1)

            # interleave: mm2 half 0 only needs batch tiles 0-3
            for bt in range(2, 7):
                emit_btile(bt)
            emit_mm2(0)
            emit_btile(7)
            emit_mm2(1)
            nc.sync.dma_start(ot_d[:], outT[:])

    _split_sync_waits(nc)
    return nc


def kernel(x, W, b, leafs):
    import ml_dtypes
    x = np.asarray(x, dtype=np.float32)
    W = np.asarray(W, dtype=np.float32)
    b = np.asarray(b, dtype=np.float32)
    leafs = np.asarray(leafs, dtype=np.float32)
    use_b = bool(np.any(b))

    # host-side layout prep (permutation / transposition / bf16 rounding)
    BF = ml_dtypes.bfloat16
    Wp = W.reshape(T, I, K)[:, _PERM, :].transpose(1, 0, 2).reshape(T * I, K)
    wt_np = np.ascontiguousarray(Wp.T.astype(BF))          # [K, 1984] bf16
    lf_np = np.ascontiguousarray(
        leafs[:, _LEAF_PERM, :].transpose(1, 0, 2).reshape(F, O))
    xT = x.T.astype(BF)                                    # [K, B] bf16
    in_maps = []
    for c in range(NCORES):
        m = {
            "xt": np.ascontiguousarray(xT[:, c * BSH:(c + 1) * BSH]),
            "wt": wt_np,
            "lf": lf_np,
        }
        if use_b:
            bp = b.reshape(T, I)[:, _PERM].T.reshape(1, T * I)
            m["bb"] = np.ascontiguousarray(bp.astype(BF))
        in_maps.append(m)

    key = ("nc", use_b)
    if key not in _CACHE:
        _CACHE[key] = _build(use_b)
    nc = _CACHE[key]

    from concourse.bass_utils import run_bass_kernel_spmd
    res = run_bass_kernel_spmd(nc, in_maps, list(range(NCORES)))
    out = np.concatenate(
        [res.results[c]["ot"].T for c in range(NCORES)], axis=0)
    return np.ascontiguousarray(out.astype(np.float32))


if __name__ == "__main__":
    rng = np.random.default_rng(0)
    ins = {
        "x": rng.standard_normal((B, K)).astype(np.float32),
        "W": (rng.standard_normal((T * I, K)) / np.sqrt(K)).astype(np.float32),
        "b": np.zeros(T * I, np.float32),
        "leafs": rng.standard_normal((T, NL, O)).astype(np.float32),
    }
    print(kernel(**ins)[:2, :4])


# revision 31
# speedup vs baseline: 1.1634x; 1.0161x over previous
"""Deep Neural Decision Forest (DNDF) forward on 8 Trainium2 NeuronCores.

Data-parallel over batch: each core gets 1024 of 8192 rows of x; W/b/leafs
replicated. Per core:
  net^T-free layout: tree_net = x @ W.T  (f32r matmuls, full PE rate)
  p = sigmoid(net) [ACT], q = 1-p [GPSIMD], both bf16
  leaf path probs via 4 levels of contiguous bf16 muls [DVE] - W rows are
  host-permuted (level-major, bit-reversed within level, tree innermost) so
  every gate read is contiguous
  routes [128b, 2048f] -> DMA-xbar transpose (bf16) -> [2048f, 1024b]
  out^T[16, 1024] = (softmax(leafs)/64)^T-ish via PE with leafs as lhsT
Host gathers the 8 out^T shards and transposes back.
"""
import sys

for _p in ("/opt/trn_rl_repo",):
    if _p not in sys.path:
        sys.path.append(_p)

import numpy as np

NCORES = 8
B, K, T, I, NL, O = 8192, 512, 64, 31, 32, 16
BSH = B // NCORES          # 1024 batch rows per core
BT = BSH // 128            # 8 batch tiles per core
NT = 4                     # n-tiles of 496 = 31*16 over the 1984 internals
NW = 496
F = T * NL                 # 2048 leaf features

_CACHE = {}


def _bitrev(x, bits):
    r = 0
    for _ in range(bits):
        r = (r << 1) | (x & 1)
        x >>= 1
    return r


def _perm_nodes():
    out = []
    for d in range(5):
        for r in range(2 ** d):
            out.append(2 ** d - 1 + _bitrev(r, d))
    return np.array(out)


_PERM = _perm_nodes()                                     # slot -> node
_LEAF_PERM = np.array([_bitrev(i, 5) for i in range(NL)])  # layout -> leaf


def _split_sync_waits(nc, cap=1):
    """This container's walrus rejects >1 sync wait per instruction; move
    excess waits onto standalone EventSemaphore insts (same-engine order
    makes this equivalent)."""
    import bass_rust
    from concourse import mybir

    n = 0
    for fn in nc.m.functions:
        for bb in fn.blocks:
            insts = bb.instructions
            out = []
            changed = False
            for ins in insts:
                si = getattr(ins, "sync_info", None)
                waits = list(si.on_wait) if si is not None and si.on_wait else []
                if len(waits) > cap:
                    changed = True
                    for k in range(0, len(waits) - cap, cap):
                        n += 1
                        out.append(mybir.InstEventSemaphore(
                            name=f"I-wsplit{n}",
                            engine=ins.engine,
                            ins=[], outs=[],
                            debug=ins.debug,
                            sync_info=bass_rust.SyncInfo(
                                on_wait=waits[k:k + cap], on_update=[]),
                        ))
                    ins.sync_info = bass_rust.SyncInfo(
                        on_wait=waits[len(waits) - cap:],
                        on_update=list(si.on_update))
                out.append(ins)
            if changed:
                insts[:] = out


def _slim_drain_and_barrier(self, tick_clock, wait_clock):
    # Tile's stock teardown is drain -> barrier -> sem-clears -> barrier
    # (~9us of EVSEM butterfly). The trailing barrier only shields the
    # gpsimd sem-clears from a *subsequent* NEFF execution, which the
    # runtime already orders on queue completion - drop it.
    import concourse.tile as tile
    nc = self.nc
    drain_inst = nc.sync.drain()
    wait_clock.add_sem_waits(
        drain_inst.ins, tile.ScopedClock({None: tick_clock.global_clock}))
    nc.all_engine_barrier()
    popped = nc._tile_sem_poison_stack.pop()
    assert popped is self._sem_poison
    nc.clear_and_free_semaphores(list(self.sems.allocated().values()))


def _build(use_b):
    import concourse.bass as bass
    import concourse.tile as tile
    from concourse import mybir

    tile.TileContext._drain_and_barrier = _slim_drain_and_barrier

    F32 = mybir.dt.float32
    F32R = mybir.dt.float32r
    BF16 = mybir.dt.bfloat16
    AF = mybir.ActivationFunctionType
    MUL = mybir.AluOpType.mult
    ADD = mybir.AluOpType.add

    nc = bass.Bass()
    xt_d = nc.dram_tensor("xt", [K, BSH], BF16, kind="ExternalInput")
    wt_d = nc.dram_tensor("wt", [K, T * I], BF16, kind="ExternalInput")
    lf_d = nc.dram_tensor("lf", [F, O], F32, kind="ExternalInput")
    if use_b:
        b_d = nc.dram_tensor("bb", [1, T * I], BF16, kind="ExternalInput")
    ot_d = nc.dram_tensor("ot", [O, BSH], F32, kind="ExternalOutput")

    with tile.TileContext(nc) as tc:
        with (
            tc.tile_pool(name="const", bufs=1) as const,
            tc.tile_pool(name="work", bufs=2) as work,
            tc.tile_pool(name="psum", bufs=8, space="PSUM") as psum,
        ):
            # ---- prologue loads (f32 bits into f32r tiles via bitcast),
            # chunked per K-tile so mm1 can start after the first pair ----
            xt = const.tile([128, 4, BSH], BF16)
            wt = const.tile([128, 4, T * I], BF16)
            for c in range(4):
                nc.sync.dma_start(
                    xt[:, c, :], xt_d[c * 128:(c + 1) * 128, :])
                for h in range(2):
                    nc.sync.dma_start(
                        wt[:, c, h * 992:(h + 1) * 992],
                        wt_d[c * 128:(c + 1) * 128, h * 992:(h + 1) * 992])
            if use_b:
                bt_s = const.tile([1, T * I], BF16)
                nc.sync.dma_start(bt_s[:], b_d[:])
                ones = const.tile([1, 128], BF16)
                nc.gpsimd.memset(ones[:], 1.0)

            # ---- leaf softmax (off critical path) ----
            lf = const.tile([128, 16, O], F32)
            nc.sync.dma_start(lf[:], lf_d.rearrange("(c k) o -> k c o", k=128))
            ex = const.tile([128, 16, O], F32)
            nc.scalar.activation(ex[:], lf[:], AF.Exp)
            sums = const.tile([128, 16], F32)
            nc.vector.reduce_sum(sums[:], ex[:], axis=mybir.AxisListType.X)
            nc.vector.tensor_scalar_mul(sums[:], sums[:], float(T))
            rs = const.tile([128, 16], F32)
            nc.vector.reciprocal(rs[:], sums[:])
            lsm = const.tile([128, 16, O], BF16)
            for c in range(16):
                nc.gpsimd.tensor_scalar(
                    lsm[:, c, :], ex[:, c, :], rs[:, c:c + 1], None, op0=MUL)

            rT = const.tile([128, 16, BSH], BF16)   # routes^T, filled per btile

            # ---- main loop over batch tiles ----
            outT = const.tile([O, BSH], F32)

            def emit_mm2(nch):
                po = psum.tile([O, 512], F32, tag="ps")
                for kcc in range(16):
                    nc.tensor.matmul(
                        po[:], lsm[:, kcc, :],
                        rT[:, kcc, nch * 512:(nch + 1) * 512],
                        start=(kcc == 0), stop=(kcc == 15))
                nc.vector.tensor_copy(
                    outT[:, nch * 512:(nch + 1) * 512], po[:])

            def emit_b_row(banks):
                for nt in range(NT):
                    nc.tensor.matmul(
                        banks[nt][:, 0:NW], ones[:],
                        bt_s[:, nt * NW:(nt + 1) * NW],
                        start=True, stop=False)

            def emit_post(bt, banks):
                # per-bank sigmoid: bank nt's p-slice is ready as soon as its
                # accumulation stops, letting the routing chain start early
                p = work.tile([128, T * I], BF16, tag="p")
                for nt in range(NT):
                    nc.scalar.activation(
                        p[:, nt * NW:(nt + 1) * NW], banks[nt][:, 0:NW],
                        AF.Sigmoid)

                # q-branches via R*(1-p) = R - R*p: only the 64-wide level-0
                # q-slice is ever materialized
                q0 = work.tile([128, T], BF16, tag="q0")
                nc.vector.tensor_scalar(
                    q0[:], p[:, 0:T], -1.0, 1.0, op0=MUL, op1=ADD)

                R1 = work.tile([128, 2, 2 * T], BF16, tag="R1")
                R2 = work.tile([128, 2, 4 * T], BF16, tag="R2")
                R3 = work.tile([128, 2, 8 * T], BF16, tag="R3")
                R4 = work.tile([128, 2, 16 * T], BF16, tag="R4")
                # level 1, split so R0 = [p0|q0] never materializes
                nc.vector.tensor_mul(R1[:, 0, 0:T], p[:, 0:T], p[:, T:2 * T])
                nc.vector.tensor_mul(R1[:, 0, T:2 * T], q0[:], p[:, 2 * T:3 * T])
                nc.vector.tensor_sub(R1[:, 1, 0:T], p[:, 0:T], R1[:, 0, 0:T])
                nc.vector.tensor_sub(R1[:, 1, T:2 * T], q0[:], R1[:, 0, T:2 * T])
                prev = R1[:].rearrange("p a c -> p (a c)")
                for (R, s0, s1) in [(R2, 3, 7), (R3, 7, 15), (R4, 15, 31)]:
                    flat = R[:].rearrange("p a c -> p (a c)")
                    nc.vector.tensor_mul(            # p-branch (1-D, bf16 2x)
                        R[:, 0, :], prev, p[:, s0 * T:s1 * T])
                    nc.vector.tensor_sub(            # q-branch = prev - p-branch
                        R[:, 1, :], prev, R[:, 0, :])
                    prev = flat                 # 1-D view of full level

                nc.sync.dma_start_transpose(
                    rT[:, :, bt * 128:(bt + 1) * 128],
                    R4[:].rearrange("p a c -> p (a c)"))

            def emit_btile(bt):
                # nt-outer with one PSUM bank per n-tile: each bank frees
                # right after its own sigmoid, and early banks' sigmoids and
                # routing levels overlap the later banks' matmuls
                xcol = xt[:, :, bt * 128:(bt + 1) * 128]
                banks = []
                for nt in range(NT):
                    pb = psum.tile([128, 512], F32, tag="ps", name=f"pb{bt}_{nt}")
                    if use_b:
                        nc.tensor.matmul(
                            pb[:, 0:NW], ones[:],
                            bt_s[:, nt * NW:(nt + 1) * NW],
                            start=True, stop=False)
                    for kc in range(4):
                        nc.tensor.matmul(
                            pb[:, 0:NW],
                            xcol[:, kc, :],
                            wt[:, kc, nt * NW:(nt + 1) * NW],
                            start=(kc == 0) and not use_b,
                            stop=(kc == 3),
                        )
                    banks.append(pb)
                emit_post(bt, banks)

            # streaming prologue: batch tiles 0-1 chase the chunked input
            # DMAs (kc-outer, W-half-inner) so PE starts as soon as the
            # first K-chunk lands
            banks0 = [psum.tile([128, 512], F32, tag="ps", name=f"pbs0_{i}") for i in range(NT)]
            banks1 = [psum.tile([128, 512], F32, tag="ps", name=f"pbs1_{i}") for i in range(NT)]
            if use_b:
                emit_b_row(banks0)
                emit_b_row(banks1)
            for kc in range(4):
                if kc < 3:
                    order = [(h, bt) for h in range(2) for bt in range(2)]
                else:
                    # de-interleave the last K-sweep: bt0 finishes (and frees
                    # its PSUM banks via sigmoids) ahead of bt1
                    order = [(h, bt) for bt in range(2) for h in range(2)]
                for h, bt in order:
                    bks = banks0 if bt == 0 else banks1
                    xcol = xt[:, kc, bt * 128:(bt + 1) * 128]
                    for nt in (2 * h, 2 * h + 1):
                        nc.tensor.matmul(
                            bks[nt][:, 0:NW], xcol,
                            wt[:, kc, nt * NW:(nt + 1) * NW],
                            start=(kc == 0) and not use_b,
                            stop=(kc == 3))
            emit_post(0, banks0)
            emit_post(1, banks1)

            # interleave: mm2 half 0 only needs batch tiles 0-3
            for bt in range(2, 7):
                emit_btile(bt)
            emit_mm2(0)
            emit_btile(7)
            emit_mm2(1)
            nc.sync.dma_start(ot_d[:], outT[:])

    _split_sync_waits(nc)
    return nc


def kernel(x, W, b, leafs):
    import ml_dtypes
    x = np.asarray(x, dtype=np.float32)
    W = np.asarray(W, dtype=np.float32)
    b = np.asarray(b, dtype=np.float32)
    leafs = np.asarray(leafs, dtype=np.float32)
    use_b = bool(np.any(b))

    # host-side layout prep (permutation / transposition / bf16 rounding)
    BF = ml_dtypes.bfloat16
    Wp = W.reshape(T, I, K)[:, _PERM, :].transpose(1, 0, 2).reshape(T * I, K)
    wt_np = np.ascontiguousarray(Wp.T.astype(BF))          # [K, 1984] bf16
    lf_np = np.ascontiguousarray(
        leafs[:, _LEAF_PERM, :].transpose(1, 0, 2).reshape(F, O))
    xT = x.T.astype(BF)                                    # [K, B] bf16
    in_maps = []
    for c in range(NCORES):
        m = {
            "xt": np.ascontiguousarray(xT[:, c * BSH:(c + 1) * BSH]),
            "wt": wt_np,
            "lf": lf_np,
        }
        if use_b:
            bp = b.reshape(T, I)[:, _PERM].T.reshape(1, T * I)
            m["bb"] = np.ascontiguousarray(bp.astype(BF))
        in_maps.append(m)

    key = ("nc", use_b)
    if key not in _CACHE:
        _CACHE[key] = _build(use_b)
    nc = _CACHE[key]

    from concourse.bass_utils import run_bass_kernel_spmd
    res = run_bass_kernel_spmd(nc, in_maps, list(range(NCORES)))
    out = np.concatenate(
        [res.results[c]["ot"].T for c in range(NCORES)], axis=0)
    return np.ascontiguousarray(out.astype(np.float32))


if __name__ == "__main__":
    rng = np.random.default_rng(0)
    ins = {
        "x": rng.standard_normal((B, K)).astype(np.float32),
        "W": (rng.standard_normal((T * I, K)) / np.sqrt(K)).astype(np.float32),
        "b": np.zeros(T * I, np.float32),
        "leafs": rng.standard_normal((T, NL, O)).astype(np.float32),
    }
    print(kernel(**ins)[:2, :4])


# revision 32
# speedup vs baseline: 1.1889x; 1.0220x over previous
"""Deep Neural Decision Forest (DNDF) forward on 8 Trainium2 NeuronCores.

Data-parallel over batch: each core gets 1024 of 8192 rows of x; W/b/leafs
replicated. Per core:
  net^T-free layout: tree_net = x @ W.T  (f32r matmuls, full PE rate)
  p = sigmoid(net) [ACT], q = 1-p [GPSIMD], both bf16
  leaf path probs via 4 levels of contiguous bf16 muls [DVE] - W rows are
  host-permuted (level-major, bit-reversed within level, tree innermost) so
  every gate read is contiguous
  routes [128b, 2048f] -> DMA-xbar transpose (bf16) -> [2048f, 1024b]
  out^T[16, 1024] = (softmax(leafs)/64)^T-ish via PE with leafs as lhsT
Host gathers the 8 out^T shards and transposes back.
"""
import sys

for _p in ("/opt/trn_rl_repo",):
    if _p not in sys.path:
        sys.path.append(_p)

import numpy as np

NCORES = 8
B, K, T, I, NL, O = 8192, 512, 64, 31, 32, 16
BSH = B // NCORES          # 1024 batch rows per core
BT = BSH // 128            # 8 batch tiles per core
NT = 4                     # n-tiles of 496 = 31*16 over the 1984 internals
NW = 496
F = T * NL                 # 2048 leaf features

_CACHE = {}


def _bitrev(x, bits):
    r = 0
    for _ in range(bits):
        r = (r << 1) | (x & 1)
        x >>= 1
    return r


def _perm_nodes():
    out = []
    for d in range(5):
        for r in range(2 ** d):
            out.append(2 ** d - 1 + _bitrev(r, d))
    return np.array(out)


_PERM = _perm_nodes()                                     # slot -> node
_LEAF_PERM = np.array([_bitrev(i, 5) for i in range(NL)])  # layout -> leaf


def _split_sync_waits(nc, cap=1):
    """This container's walrus rejects >1 sync wait per instruction; move
    excess waits onto standalone EventSemaphore insts (same-engine order
    makes this equivalent)."""
    import bass_rust
    from concourse import mybir

    n = 0
    for fn in nc.m.functions:
        for bb in fn.blocks:
            insts = bb.instructions
            out = []
            changed = False
            for ins in insts:
                si = getattr(ins, "sync_info", None)
                waits = list(si.on_wait) if si is not None and si.on_wait else []
                if len(waits) > cap:
                    changed = True
                    for k in range(0, len(waits) - cap, cap):
                        n += 1
                        out.append(mybir.InstEventSemaphore(
                            name=f"I-wsplit{n}",
                            engine=ins.engine,
                            ins=[], outs=[],
                            debug=ins.debug,
                            sync_info=bass_rust.SyncInfo(
                                on_wait=waits[k:k + cap], on_update=[]),
                        ))
                    ins.sync_info = bass_rust.SyncInfo(
                        on_wait=waits[len(waits) - cap:],
                        on_update=list(si.on_update))
                out.append(ins)
            if changed:
                insts[:] = out


def _slim_drain_and_barrier(self, tick_clock, wait_clock):
    # Tile's stock teardown is drain -> barrier -> sem-clears -> barrier
    # (~9us of EVSEM butterfly). The trailing barrier only shields the
    # gpsimd sem-clears from a *subsequent* NEFF execution, which the
    # runtime already orders on queue completion - drop it.
    import concourse.tile as tile
    nc = self.nc
    drain_inst = nc.sync.drain()
    wait_clock.add_sem_waits(
        drain_inst.ins, tile.ScopedClock({None: tick_clock.global_clock}))
    nc.all_engine_barrier(sem_only=True)
    popped = nc._tile_sem_poison_stack.pop()
    assert popped is self._sem_poison
    nc.clear_and_free_semaphores(list(self.sems.allocated().values()))


def _build(use_b):
    import concourse.bass as bass
    import concourse.tile as tile
    from concourse import mybir

    tile.TileContext._drain_and_barrier = _slim_drain_and_barrier

    F32 = mybir.dt.float32
    F32R = mybir.dt.float32r
    BF16 = mybir.dt.bfloat16
    AF = mybir.ActivationFunctionType
    MUL = mybir.AluOpType.mult
    ADD = mybir.AluOpType.add

    nc = bass.Bass()
    xt_d = nc.dram_tensor("xt", [K, BSH], BF16, kind="ExternalInput")
    wt_d = nc.dram_tensor("wt", [K, T * I], BF16, kind="ExternalInput")
    lf_d = nc.dram_tensor("lf", [F, O], F32, kind="ExternalInput")
    if use_b:
        b_d = nc.dram_tensor("bb", [1, T * I], BF16, kind="ExternalInput")
    ot_d = nc.dram_tensor("ot", [O, BSH], F32, kind="ExternalOutput")

    with tile.TileContext(nc) as tc:
        with (
            tc.tile_pool(name="const", bufs=1) as const,
            tc.tile_pool(name="work", bufs=2) as work,
            tc.tile_pool(name="psum", bufs=8, space="PSUM") as psum,
        ):
            # ---- prologue loads (f32 bits into f32r tiles via bitcast),
            # chunked per K-tile so mm1 can start after the first pair ----
            xt = const.tile([128, 4, BSH], BF16)
            wt = const.tile([128, 4, T * I], BF16)
            for c in range(4):
                nc.sync.dma_start(
                    xt[:, c, :], xt_d[c * 128:(c + 1) * 128, :])
                for h in range(2):
                    nc.sync.dma_start(
                        wt[:, c, h * 992:(h + 1) * 992],
                        wt_d[c * 128:(c + 1) * 128, h * 992:(h + 1) * 992])
            if use_b:
                bt_s = const.tile([1, T * I], BF16)
                nc.sync.dma_start(bt_s[:], b_d[:])
                ones = const.tile([1, 128], BF16)
                nc.gpsimd.memset(ones[:], 1.0)

            # ---- leaf softmax (off critical path) ----
            lf = const.tile([128, 16, O], F32)
            nc.sync.dma_start(lf[:], lf_d.rearrange("(c k) o -> k c o", k=128))
            ex = const.tile([128, 16, O], F32)
            nc.scalar.activation(ex[:], lf[:], AF.Exp)
            sums = const.tile([128, 16], F32)
            nc.vector.reduce_sum(sums[:], ex[:], axis=mybir.AxisListType.X)
            nc.vector.tensor_scalar_mul(sums[:], sums[:], float(T))
            rs = const.tile([128, 16], F32)
            nc.vector.reciprocal(rs[:], sums[:])
            lsm = const.tile([128, 16, O], BF16)
            for c in range(16):
                nc.gpsimd.tensor_scalar(
                    lsm[:, c, :], ex[:, c, :], rs[:, c:c + 1], None, op0=MUL)

            rT = const.tile([128, 16, BSH], BF16)   # routes^T, filled per btile

            # ---- main loop over batch tiles ----
            outT = const.tile([O, BSH], F32)

            def emit_mm2(nch):
                po = psum.tile([O, 512], F32, tag="ps")
                for kcc in range(16):
                    nc.tensor.matmul(
                        po[:], lsm[:, kcc, :],
                        rT[:, kcc, nch * 512:(nch + 1) * 512],
                        start=(kcc == 0), stop=(kcc == 15))
                nc.vector.tensor_copy(
                    outT[:, nch * 512:(nch + 1) * 512], po[:])

            def emit_b_row(banks):
                for nt in range(NT):
                    nc.tensor.matmul(
                        banks[nt][:, 0:NW], ones[:],
                        bt_s[:, nt * NW:(nt + 1) * NW],
                        start=True, stop=False)

            def emit_post(bt, banks):
                # per-bank sigmoid: bank nt's p-slice is ready as soon as its
                # accumulation stops, letting the routing chain start early
                p = work.tile([128, T * I], BF16, tag="p")
                for nt in range(NT):
                    nc.scalar.activation(
                        p[:, nt * NW:(nt + 1) * NW], banks[nt][:, 0:NW],
                        AF.Sigmoid)

                # q-branches via R*(1-p) = R - R*p: only the 64-wide level-0
                # q-slice is ever materialized
                q0 = work.tile([128, T], BF16, tag="q0")
                nc.vector.tensor_scalar(
                    q0[:], p[:, 0:T], -1.0, 1.0, op0=MUL, op1=ADD)

                R1 = work.tile([128, 2, 2 * T], BF16, tag="R1")
                R2 = work.tile([128, 2, 4 * T], BF16, tag="R2")
                R3 = work.tile([128, 2, 8 * T], BF16, tag="R3")
                R4 = work.tile([128, 2, 16 * T], BF16, tag="R4")
                # level 1, split so R0 = [p0|q0] never materializes
                nc.vector.tensor_mul(R1[:, 0, 0:T], p[:, 0:T], p[:, T:2 * T])
                nc.vector.tensor_mul(R1[:, 0, T:2 * T], q0[:], p[:, 2 * T:3 * T])
                nc.vector.tensor_sub(R1[:, 1, 0:T], p[:, 0:T], R1[:, 0, 0:T])
                nc.vector.tensor_sub(R1[:, 1, T:2 * T], q0[:], R1[:, 0, T:2 * T])
                prev = R1[:].rearrange("p a c -> p (a c)")
                for (R, s0, s1) in [(R2, 3, 7), (R3, 7, 15), (R4, 15, 31)]:
                    flat = R[:].rearrange("p a c -> p (a c)")
                    nc.vector.tensor_mul(            # p-branch (1-D, bf16 2x)
                        R[:, 0, :], prev, p[:, s0 * T:s1 * T])
                    nc.vector.tensor_sub(            # q-branch = prev - p-branch
                        R[:, 1, :], prev, R[:, 0, :])
                    prev = flat                 # 1-D view of full level

                if bt == BT - 1:
                    # final tile: transpose in halves so mm2's first 8
                    # K-chunks start while the q-half is still finishing
                    nc.sync.dma_start_transpose(
                        rT[:, 0:8, bt * 128:(bt + 1) * 128], R4[:, 0, :])
                    nc.sync.dma_start_transpose(
                        rT[:, 8:16, bt * 128:(bt + 1) * 128], R4[:, 1, :])
                else:
                    nc.sync.dma_start_transpose(
                        rT[:, :, bt * 128:(bt + 1) * 128],
                        R4[:].rearrange("p a c -> p (a c)"))

            def emit_btile(bt):
                # nt-outer with one PSUM bank per n-tile: each bank frees
                # right after its own sigmoid, and early banks' sigmoids and
                # routing levels overlap the later banks' matmuls
                xcol = xt[:, :, bt * 128:(bt + 1) * 128]
                banks = []
                for nt in range(NT):
                    pb = psum.tile([128, 512], F32, tag="ps", name=f"pb{bt}_{nt}")
                    if use_b:
                        nc.tensor.matmul(
                            pb[:, 0:NW], ones[:],
                            bt_s[:, nt * NW:(nt + 1) * NW],
                            start=True, stop=False)
                    for kc in range(4):
                        nc.tensor.matmul(
                            pb[:, 0:NW],
                            xcol[:, kc, :],
                            wt[:, kc, nt * NW:(nt + 1) * NW],
                            start=(kc == 0) and not use_b,
                            stop=(kc == 3),
                        )
                    banks.append(pb)
                emit_post(bt, banks)

            # streaming prologue: batch tiles 0-1 chase the chunked input
            # DMAs (kc-outer, W-half-inner) so PE starts as soon as the
            # first K-chunk lands
            banks0 = [psum.tile([128, 512], F32, tag="ps", name=f"pbs0_{i}") for i in range(NT)]
            banks1 = [psum.tile([128, 512], F32, tag="ps", name=f"pbs1_{i}") for i in range(NT)]
            if use_b:
                emit_b_row(banks0)
                emit_b_row(banks1)
            for kc in range(4):
                if kc < 3:
                    order = [(h, bt) for h in range(2) for bt in range(2)]
                else:
                    # de-interleave the last K-sweep: bt0 finishes (and frees
                    # its PSUM banks via sigmoids) ahead of bt1
                    order = [(h, bt) for bt in range(2) for h in range(2)]
                for h, bt in order:
                    bks = banks0 if bt == 0 else banks1
                    xcol = xt[:, kc, bt * 128:(bt + 1) * 128]
                    for nt in (2 * h, 2 * h + 1):
                        nc.tensor.matmul(
                            bks[nt][:, 0:NW], xcol,
                            wt[:, kc, nt * NW:(nt + 1) * NW],
                            start=(kc == 0) and not use_b,
                            stop=(kc == 3))
            emit_post(0, banks0)
            emit_post(1, banks1)

            # interleave: mm2 half 0 only needs batch tiles 0-3
            for bt in range(2, 7):
                emit_btile(bt)
            emit_mm2(0)
            emit_btile(7)
            emit_mm2(1)
            nc.sync.dma_start(ot_d[:], outT[:])

    _split_sync_waits(nc)
    return nc


def kernel(x, W, b, leafs):
    import ml_dtypes
    x = np.asarray(x, dtype=np.float32)
    W = np.asarray(W, dtype=np.float32)
    b = np.asarray(b, dtype=np.float32)
    leafs = np.asarray(leafs, dtype=np.float32)
    use_b = bool(np.any(b))

    # host-side layout prep (permutation / transposition / bf16 rounding)
    BF = ml_dtypes.bfloat16
    Wp = W.reshape(T, I, K)[:, _PERM, :].transpose(1, 0, 2).reshape(T * I, K)
    wt_np = np.ascontiguousarray(Wp.T.astype(BF))          # [K, 1984] bf16
    lf_np = np.ascontiguousarray(
        leafs[:, _LEAF_PERM, :].transpose(1, 0, 2).reshape(F, O))
    xT = x.T.astype(BF)                                    # [K, B] bf16
    in_maps = []
    for c in range(NCORES):
        m = {
            "xt": np.ascontiguousarray(xT[:, c * BSH:(c + 1) * BSH]),
            "wt": wt_np,
            "lf": lf_np,
        }
        if use_b:
            bp = b.reshape(T, I)[:, _PERM].T.reshape(1, T * I)
            m["bb"] = np.ascontiguousarray(bp.astype(BF))
        in_maps.append(m)

    key = ("nc", use_b)
    if key not in _CACHE:
        _CACHE[key] = _build(use_b)
    nc = _CACHE[key]

    from concourse.bass_utils import run_bass_kernel_spmd
    res = run_bass_kernel_spmd(nc, in_maps, list(range(NCORES)))
    out = np.concatenate(
        [res.results[c]["ot"].T for c in range(NCORES)], axis=0)
    return np.ascontiguousarray(out.astype(np.float32))


if __name__ == "__main__":
    rng = np.random.default_rng(0)
    ins = {
        "x": rng.standard_normal((B, K)).astype(np.float32),
        "W": (rng.standard_normal((T * I, K)) / np.sqrt(K)).astype(np.float32),
        "b": np.zeros(T * I, np.float32),
        "leafs": rng.standard_normal((T, NL, O)).astype(np.float32),
    }
    print(kernel(**ins)[:2, :4])


# revision 33
# speedup vs baseline: 1.2688x; 1.0672x over previous
"""Deep Neural Decision Forest (DNDF) forward on 8 Trainium2 NeuronCores.

Data-parallel over batch: each core gets 1024 of 8192 rows of x; W/b/leafs
replicated. Per core:
  net^T-free layout: tree_net = x @ W.T  (f32r matmuls, full PE rate)
  p = sigmoid(net) [ACT], q = 1-p [GPSIMD], both bf16
  leaf path probs via 4 levels of contiguous bf16 muls [DVE] - W rows are
  host-permuted (level-major, bit-reversed within level, tree innermost) so
  every gate read is contiguous
  routes [128b, 2048f] -> DMA-xbar transpose (bf16) -> [2048f, 1024b]
  out^T[16, 1024] = (softmax(leafs)/64)^T-ish via PE with leafs as lhsT
Host gathers the 8 out^T shards and transposes back.
"""
import sys

for _p in ("/opt/trn_rl_repo",):
    if _p not in sys.path:
        sys.path.append(_p)

import numpy as np

NCORES = 8
B, K, T, I, NL, O = 8192, 512, 64, 31, 32, 16
BSH = B // NCORES          # 1024 batch rows per core
BT = BSH // 128            # 8 batch tiles per core
NT = 4                     # n-tiles of 496 = 31*16 over the 1984 internals
NW = 496
F = T * NL                 # 2048 leaf features

_CACHE = {}


def _bitrev(x, bits):
    r = 0
    for _ in range(bits):
        r = (r << 1) | (x & 1)
        x >>= 1
    return r


def _perm_nodes():
    out = []
    for d in range(5):
        for r in range(2 ** d):
            out.append(2 ** d - 1 + _bitrev(r, d))
    return np.array(out)


_PERM = _perm_nodes()                                     # slot -> node
_LEAF_PERM = np.array([_bitrev(i, 5) for i in range(NL)])  # layout -> leaf


def _split_sync_waits(nc, cap=1):
    """This container's walrus rejects >1 sync wait per instruction; move
    excess waits onto standalone EventSemaphore insts (same-engine order
    makes this equivalent)."""
    import bass_rust
    from concourse import mybir

    n = 0
    for fn in nc.m.functions:
        for bb in fn.blocks:
            insts = bb.instructions
            out = []
            changed = False
            for ins in insts:
                si = getattr(ins, "sync_info", None)
                waits = list(si.on_wait) if si is not None and si.on_wait else []
                if len(waits) > cap:
                    changed = True
                    for k in range(0, len(waits) - cap, cap):
                        n += 1
                        out.append(mybir.InstEventSemaphore(
                            name=f"I-wsplit{n}",
                            engine=ins.engine,
                            ins=[], outs=[],
                            debug=ins.debug,
                            sync_info=bass_rust.SyncInfo(
                                on_wait=waits[k:k + cap], on_update=[]),
                        ))
                    ins.sync_info = bass_rust.SyncInfo(
                        on_wait=waits[len(waits) - cap:],
                        on_update=list(si.on_update))
                out.append(ins)
            if changed:
                insts[:] = out


def _slim_drain_and_barrier(self, tick_clock, wait_clock):
    # Tile's stock teardown is drain -> barrier -> sem-clears -> barrier
    # (~9us of EVSEM butterfly). The trailing barrier only shields the
    # gpsimd sem-clears from a *subsequent* NEFF execution, which the
    # runtime already orders on queue completion - drop it.
    import concourse.tile as tile
    nc = self.nc
    drain_inst = nc.sync.drain()
    wait_clock.add_sem_waits(
        drain_inst.ins, tile.ScopedClock({None: tick_clock.global_clock}))
    nc.all_engine_barrier(sem_only=True)
    popped = nc._tile_sem_poison_stack.pop()
    assert popped is self._sem_poison
    nc.clear_and_free_semaphores(list(self.sems.allocated().values()))


def _build(use_b):
    import concourse.bass as bass
    import concourse.tile as tile
    from concourse import mybir

    tile.TileContext._drain_and_barrier = _slim_drain_and_barrier

    F32 = mybir.dt.float32
    F32R = mybir.dt.float32r
    BF16 = mybir.dt.bfloat16
    AF = mybir.ActivationFunctionType
    MUL = mybir.AluOpType.mult
    ADD = mybir.AluOpType.add

    nc = bass.Bass()
    xt_d = nc.dram_tensor("xt", [K, BSH], BF16, kind="ExternalInput")
    wt_d = nc.dram_tensor("wt", [K, T * I], BF16, kind="ExternalInput")
    lf_d = nc.dram_tensor("lf", [F, O], F32, kind="ExternalInput")
    if use_b:
        b_d = nc.dram_tensor("bb", [1, T * I], BF16, kind="ExternalInput")
    ot_d = nc.dram_tensor("ot", [O, BSH], F32, kind="ExternalOutput")

    with tile.TileContext(nc) as tc:
        with (
            tc.tile_pool(name="const", bufs=1) as const,
            tc.tile_pool(name="work", bufs=3) as work,
            tc.tile_pool(name="psum", bufs=8, space="PSUM") as psum,
        ):
            # ---- prologue loads (f32 bits into f32r tiles via bitcast),
            # chunked per K-tile so mm1 can start after the first pair ----
            xt = const.tile([128, 4, BSH], BF16)
            wt = const.tile([128, 4, T * I], BF16)
            for c in range(4):
                nc.sync.dma_start(
                    xt[:, c, :], xt_d[c * 128:(c + 1) * 128, :])
                for h in range(2):
                    nc.sync.dma_start(
                        wt[:, c, h * 992:(h + 1) * 992],
                        wt_d[c * 128:(c + 1) * 128, h * 992:(h + 1) * 992])
            if use_b:
                bt_s = const.tile([1, T * I], BF16)
                nc.sync.dma_start(bt_s[:], b_d[:])
                ones = const.tile([1, 128], BF16)
                nc.gpsimd.memset(ones[:], 1.0)

            # ---- leaf softmax (off critical path) ----
            lf = const.tile([128, 16, O], F32)
            nc.sync.dma_start(lf[:], lf_d.rearrange("(c k) o -> k c o", k=128))
            ex = const.tile([128, 16, O], F32)
            nc.scalar.activation(ex[:], lf[:], AF.Exp)
            sums = const.tile([128, 16], F32)
            nc.vector.reduce_sum(sums[:], ex[:], axis=mybir.AxisListType.X)
            nc.vector.tensor_scalar_mul(sums[:], sums[:], float(T))
            rs = const.tile([128, 16], F32)
            nc.vector.reciprocal(rs[:], sums[:])
            lsm = const.tile([128, 16, O], BF16)
            for c in range(16):
                nc.gpsimd.tensor_scalar(
                    lsm[:, c, :], ex[:, c, :], rs[:, c:c + 1], None, op0=MUL)

            rT = const.tile([128, 16, BSH], BF16)   # routes^T, filled per btile

            # ---- main loop over batch tiles ----
            outT = const.tile([O, BSH], F32)

            def emit_mm2(nch):
                po = psum.tile([O, 512], F32, tag="ps")
                for kcc in range(16):
                    nc.tensor.matmul(
                        po[:], lsm[:, kcc, :],
                        rT[:, kcc, nch * 512:(nch + 1) * 512],
                        start=(kcc == 0), stop=(kcc == 15))
                nc.vector.tensor_copy(
                    outT[:, nch * 512:(nch + 1) * 512], po[:])

            def emit_b_row(banks):
                for nt in range(NT):
                    nc.tensor.matmul(
                        banks[nt][:, 0:NW], ones[:],
                        bt_s[:, nt * NW:(nt + 1) * NW],
                        start=True, stop=False)

            def emit_post(bt, banks):
                # per-bank sigmoid: bank nt's p-slice is ready as soon as its
                # accumulation stops, letting the routing chain start early
                p = work.tile([128, T * I], BF16, tag="p")
                for nt in range(NT):
                    nc.scalar.activation(
                        p[:, nt * NW:(nt + 1) * NW], banks[nt][:, 0:NW],
                        AF.Sigmoid)

                # q-branches via R*(1-p) = R - R*p: only the 64-wide level-0
                # q-slice is ever materialized
                q0 = work.tile([128, T], BF16, tag="q0")
                nc.vector.tensor_scalar(
                    q0[:], p[:, 0:T], -1.0, 1.0, op0=MUL, op1=ADD)

                R1 = work.tile([128, 2, 2 * T], BF16, tag="R1")
                R2 = work.tile([128, 2, 4 * T], BF16, tag="R2")
                R3 = work.tile([128, 2, 8 * T], BF16, tag="R3")
                R4 = work.tile([128, 2, 16 * T], BF16, tag="R4")
                # level 1, split so R0 = [p0|q0] never materializes
                nc.vector.tensor_mul(R1[:, 0, 0:T], p[:, 0:T], p[:, T:2 * T])
                nc.vector.tensor_mul(R1[:, 0, T:2 * T], q0[:], p[:, 2 * T:3 * T])
                nc.vector.tensor_sub(R1[:, 1, 0:T], p[:, 0:T], R1[:, 0, 0:T])
                nc.vector.tensor_sub(R1[:, 1, T:2 * T], q0[:], R1[:, 0, T:2 * T])
                prev = R1[:].rearrange("p a c -> p (a c)")
                for (R, s0, s1) in [(R2, 3, 7), (R3, 7, 15), (R4, 15, 31)]:
                    flat = R[:].rearrange("p a c -> p (a c)")
                    nc.vector.tensor_mul(            # p-branch (1-D, bf16 2x)
                        R[:, 0, :], prev, p[:, s0 * T:s1 * T])
                    nc.vector.tensor_sub(            # q-branch = prev - p-branch
                        R[:, 1, :], prev, R[:, 0, :])
                    prev = flat                 # 1-D view of full level

                if bt == BT - 1:
                    # final tile: transpose in halves so mm2's first 8
                    # K-chunks start while the q-half is still finishing
                    nc.sync.dma_start_transpose(
                        rT[:, 0:8, bt * 128:(bt + 1) * 128], R4[:, 0, :])
                    nc.sync.dma_start_transpose(
                        rT[:, 8:16, bt * 128:(bt + 1) * 128], R4[:, 1, :])
                else:
                    nc.sync.dma_start_transpose(
                        rT[:, :, bt * 128:(bt + 1) * 128],
                        R4[:].rearrange("p a c -> p (a c)"))

            def emit_btile(bt):
                # nt-outer with one PSUM bank per n-tile: each bank frees
                # right after its own sigmoid, and early banks' sigmoids and
                # routing levels overlap the later banks' matmuls
                xcol = xt[:, :, bt * 128:(bt + 1) * 128]
                banks = []
                for nt in range(NT):
                    pb = psum.tile([128, 512], F32, tag="ps", name=f"pb{bt}_{nt}")
                    if use_b:
                        nc.tensor.matmul(
                            pb[:, 0:NW], ones[:],
                            bt_s[:, nt * NW:(nt + 1) * NW],
                            start=True, stop=False)
                    for kc in range(4):
                        nc.tensor.matmul(
                            pb[:, 0:NW],
                            xcol[:, kc, :],
                            wt[:, kc, nt * NW:(nt + 1) * NW],
                            start=(kc == 0) and not use_b,
                            stop=(kc == 3),
                        )
                    banks.append(pb)
                emit_post(bt, banks)

            # streaming prologue: batch tiles 0-1 chase the chunked input
            # DMAs (kc-outer, W-half-inner) so PE starts as soon as the
            # first K-chunk lands
            banks0 = [psum.tile([128, 512], F32, tag="ps", name=f"pbs0_{i}") for i in range(NT)]
            banks1 = [psum.tile([128, 512], F32, tag="ps", name=f"pbs1_{i}") for i in range(NT)]
            if use_b:
                emit_b_row(banks0)
                emit_b_row(banks1)
            for kc in range(4):
                if kc < 3:
                    order = [(h, bt) for h in range(2) for bt in range(2)]
                else:
                    # de-interleave the last K-sweep: bt0 finishes (and frees
                    # its PSUM banks via sigmoids) ahead of bt1
                    order = [(h, bt) for bt in range(2) for h in range(2)]
                for h, bt in order:
                    bks = banks0 if bt == 0 else banks1
                    xcol = xt[:, kc, bt * 128:(bt + 1) * 128]
                    for nt in (2 * h, 2 * h + 1):
                        nc.tensor.matmul(
                            bks[nt][:, 0:NW], xcol,
                            wt[:, kc, nt * NW:(nt + 1) * NW],
                            start=(kc == 0) and not use_b,
                            stop=(kc == 3))
            emit_post(0, banks0)
            emit_post(1, banks1)

            # interleave: mm2 half 0 only needs batch tiles 0-3
            for bt in range(2, 7):
                emit_btile(bt)
            emit_mm2(0)
            emit_btile(7)
            # keep-warm dribble: dep-free matmuls fill the PE gap while
            # batch-tile 7's routing chain drains, so mm2 runs at 2.4 GHz
            dummy = psum.tile([128, 512], F32, tag="ps", name="dummy")
            for _ in range(6):
                nc.tensor.matmul(dummy[:, 0:NW], wt[:, 0, 0:128],
                                 wt[:, 0, 0:NW], start=True, stop=True)
            emit_mm2(1)
            nc.sync.dma_start(ot_d[:], outT[:])

    _split_sync_waits(nc)
    return nc


def kernel(x, W, b, leafs):
    import ml_dtypes
    x = np.asarray(x, dtype=np.float32)
    W = np.asarray(W, dtype=np.float32)
    b = np.asarray(b, dtype=np.float32)
    leafs = np.asarray(leafs, dtype=np.float32)
    use_b = bool(np.any(b))

    # host-side layout prep (permutation / transposition / bf16 rounding)
    BF = ml_dtypes.bfloat16
    Wp = W.reshape(T, I, K)[:, _PERM, :].transpose(1, 0, 2).reshape(T * I, K)
    wt_np = np.ascontiguousarray(Wp.T.astype(BF))          # [K, 1984] bf16
    lf_np = np.ascontiguousarray(
        leafs[:, _LEAF_PERM, :].transpose(1, 0, 2).reshape(F, O))
    xT = x.T.astype(BF)                                    # [K, B] bf16
    in_maps = []
    for c in range(NCORES):
        m = {
            "xt": np.ascontiguousarray(xT[:, c * BSH:(c + 1) * BSH]),
            "wt": wt_np,
            "lf": lf_np,
        }
        if use_b:
            bp = b.reshape(T, I)[:, _PERM].T.reshape(1, T * I)
            m["bb"] = np.ascontiguousarray(bp.astype(BF))
        in_maps.append(m)

    key = ("nc", use_b)
    if key not in _CACHE:
        _CACHE[key] = _build(use_b)
    nc = _CACHE[key]

    from concourse.bass_utils import run_bass_kernel_spmd
    res = run_bass_kernel_spmd(nc, in_maps, list(range(NCORES)))
    out = np.concatenate(
        [res.results[c]["ot"].T for c in range(NCORES)], axis=0)
    return np.ascontiguousarray(out.astype(np.float32))


if __name__ == "__main__":
    rng = np.random.default_rng(0)
    ins = {
        "x": rng.standard_normal((B, K)).astype(np.float32),
        "W": (rng.standard_normal((T * I, K)) / np.sqrt(K)).astype(np.float32),
        "b": np.zeros(T * I, np.float32),
        "leafs": rng.standard_normal((T, NL, O)).astype(np.float32),
    }
    print(kernel(**ins)[:2, :4])
